# revision 55
# baseline (speedup 1.0000x reference)
"""CGRUCell Trainium2 kernel — 2 batch-groups x 4-way unit split on 8
NeuronCores, Gauss 3-multiplication complex matmuls, mixed precision.

Reference semantics (conjugate cat form), per gate with weights (W, V):
  preact_real = Xr@Wr + Xi@Wi + Hr@Vr + Hi@Vi
  preact_imag = Xi@Wr - Xr@Wi + Hi@Vr - Hr@Vi
Gauss: P1 = Xr@Wr + Hr@Vr; P2 = Xi@Wi + Hi@Vi;
       P3 = (Xi-Xr)@(Wr+Wi) + (Hi-Hr)@(Vr+Vi)
       real = P1+P2, imag = P3+P1-P2  (3 matmul streams instead of 4).

Core c: batch group g=c//4 (512 rows), unit quarter p=c%4 (unit cols
[p*512:(p+1)*512] real + matching imag). Per-core MACs 9.7e9 vs 1.29e10
non-Gauss.

Precision (validated vs reference in numpy): gate r fp8e4m3 DoubleRow
(error attenuated by hard_sigmoid's 0.2 slope then averaged by the
candidate matmul; rel err 1.6e-2 vs the 2e-2 gate); gates z, h fp16
(6e-4). R_FP8=False falls back to fp16 everywhere.

Gate r runs output-transposed (weights stationary, acts moving) so r*h
emerges K-major for the 4-way AllGather feeding gate h's recurrent side.
Gates z/h run batch-major (acts stationary, weights moving) in two
m-pair passes (Gauss needs 3 PSUM banks per output tile; 8-bank PSUM
fits 6 = 3 kinds x 2 m-subtiles, so weights stream twice). Gate order
r -> z -> h hides the collective under z. hard_sigmoid runs on the
scalar engine as w = Relu(1 - Relu(0.2y + b')) (= 1-z; blend uses
h - w*(h-hh)).
"""

import sys

for _p in ("/opt/trn_rl_repo", "/root/.axon_site/_ro/trn_rl_repo"):
    if _p not in sys.path:
        sys.path.append(_p)

import numpy as np
import ml_dtypes

import concourse.bass as bass
import concourse.mybir as mybir
import concourse.tile as tile
from concourse import bacc
from concourse.bass_utils import run_bass_kernel_spmd

P = 128
U = 2048            # UNITS
B = 1024
N_CORES = 8
GROUPS = 2          # batch groups
NPAR = 4            # unit-split ways
BC = B // GROUPS    # 512 batch rows per core
MSUB = BC // P      # 4 m-subtiles
UC = U // NPAR      # 512 own unit columns (per complex half)
CCH = UC // P       # 4 col chunks of 128
KCH = U // P        # 16 k-chunks per complex half
F32 = mybir.dt.float32
F16 = mybir.dt.float16
F8 = mybir.dt.float8e4
NF16 = np.float16
NF8 = ml_dtypes.float8_e4m3

R_FP8 = True        # gate r in fp8 DoubleRow (False -> fp16, safer/slower)

import os

SKIP_COLL = bool(os.environ.get("SKIP_COLL"))  # bench A/B only: replaces
# the AllGather with local DMA loopback (WRONG results, same buffer deps)
# AllGather issue strategy (HW-measured steady-state):
#   0 = one 1MB->4MB gather at r end            (211 us)
#   1 = per-cc-pair halves issued mid-gate-r    (243 us: the collective
#       rides the same SDMA engines as gate r's weight/act streams)
#   2 = both halves back-to-back at r end       (194 us: the two
#       collectives overlap each other, halving exposed gather time)
SPLIT_GATHER = int(os.environ.get("SPLIT_GATHER", "2") or "0")

_CACHE = {}


def _build_nc(repeat=1, loop_iters=1):
    nc = bacc.Bacc(None, target_bir_lowering=False)
    AF = mybir.ActivationFunctionType
    DR = mybir.MatmulPerfMode.DoubleRow

    rdt = F8 if R_FP8 else F16
    # K-major activations [p, o, b]: value [o*128+p, b] of act.T
    xr8 = nc.dram_tensor("xr8", [P, KCH, BC], rdt, kind="ExternalInput")
    xi8 = nc.dram_tensor("xi8", [P, KCH, BC], rdt, kind="ExternalInput")
    xd8 = nc.dram_tensor("xd8", [P, KCH, BC], rdt, kind="ExternalInput")
    hr8 = nc.dram_tensor("hr8", [P, KCH, BC], rdt, kind="ExternalInput")
    hi8 = nc.dram_tensor("hi8", [P, KCH, BC], rdt, kind="ExternalInput")
    hd8 = nc.dram_tensor("hd8", [P, KCH, BC], rdt, kind="ExternalInput")
    xr16 = nc.dram_tensor("xr16", [P, KCH, BC], F16, kind="ExternalInput")
    xi16 = nc.dram_tensor("xi16", [P, KCH, BC], F16, kind="ExternalInput")
    hr16 = nc.dram_tensor("hr16", [P, KCH, BC], F16, kind="ExternalInput")
    hi16 = nc.dram_tensor("hi16", [P, KCH, BC], F16, kind="ExternalInput")
    # h own cols transposed K-major [p, cc(4 re + 4 im), b] fp16
    hT16 = nc.dram_tensor("hT16", [P, 2 * CCH, BC], F16, kind="ExternalInput")
    # h own cols batch-major [p, m, c(512 re | 512 im)] fp16
    hbm16 = nc.dram_tensor("hbm16", [P, MSUB, 2 * UC], F16,
                           kind="ExternalInput")
    wnames = ["kr", "ki", "ks", "rr", "ri", "rs"]
    # r weights ccp-major so each cc-pair's 256-col slab is contiguous
    # (4 KB/partition DMA runs instead of 256 B)
    wr = {n: nc.dram_tensor(f"wr_{n}", [CCH // 2, P, KCH, 2 * P], rdt,
                            kind="ExternalInput") for n in wnames}
    wz = {n: nc.dram_tensor(f"wz_{n}", [P, KCH, UC], F16,
                            kind="ExternalInput") for n in wnames}
    wh = {n: nc.dram_tensor(f"wh_{n}", [P, KCH, UC], F16,
                            kind="ExternalInput") for n in wnames}
    # biases [2, UC]: row0 = real bias (z ships b+2.5, hard-sigmoid
    # fold), row1 = b_im - b_re (for the saved imag diff); r pre-scaled
    # 0.2b+0.5 column-major [p, cc]
    bz = nc.dram_tensor("bz", [2, UC], F32, kind="ExternalInput")
    bh = nc.dram_tensor("bh", [2, UC], F32, kind="ExternalInput")
    brT = nc.dram_tensor("brT", [P, 2 * CCH], F32, kind="ExternalInput")
    out = nc.dram_tensor("out", [BC, 2 * UC], F16, kind="ExternalOutput")
    out_r = out.rearrange("(m p) c -> p m c", p=P)

    with tile.TileContext(nc) as tc:
        with (
            tc.tile_pool(name="acts", bufs=1) as acts,
            tc.tile_pool(name="psum", bufs=1, space="PSUM") as psum,
            tc.tile_pool(name="small", bufs=1) as small,
            tc.tile_pool(name="yp", bufs=2) as yp,
            tc.tile_pool(name="dram", bufs=1, space="DRAM") as dram,
        ):
            bt = {}
            for gn, src in (("z", bz), ("h", bh)):
                for half in range(2):
                    t = small.tile([P, UC], F32, tag=f"bt{gn}{half}",
                                   name=f"bt{gn}{half}")
                    nc.sync.dma_start(
                        t[:], src[None, half, :].to_broadcast((P, UC))
                    )
                    bt[(gn, half)] = t
            brTs = small.tile([P, 2 * CCH], F32, tag="brTs", name="brTs")
            nc.sync.dma_start(brTs[:], brT[:])
            a8 = {}
            a16 = {}
            rws = {}
            hTs = None

            _a8srcs = {"xr8": xr8, "xi8": xi8, "xd8": xd8, "hr8": hr8,
                       "hi8": hi8, "hd8": hd8}

            def load_acts_early(tag_sfx):
                # r-gate inputs whose tags are free right after gate r
                # (xr8/hr8/xi8) — issued during the PREVIOUS rep's z/h
                # phase so the next rep's first matmul chains never wait.
                # SP-ring, gate-r phase-consumption order (kr: xr8,
                # rr: hr8, ki: xi8), two half-loads each so the first
                # chain starts at half-landing.
                h = KCH // 2
                for nm in ("xr8", "hr8", "xi8"):
                    a8[nm] = acts.tile([P, KCH, BC], rdt, tag=nm,
                                       name=f"{nm}_{tag_sfx}")
                    nc.sync.dma_start(a8[nm][:, :h, :],
                                      _a8srcs[nm][:, :h, :])
                    nc.sync.dma_start(a8[nm][:, h:, :],
                                      _a8srcs[nm][:, h:, :])

            def load_acts():
                # rest of the act loads: hi8/xd8/hd8 (their tags alias the
                # previous rep's z/h output buffers, so they wait for its
                # blend), hTs, and the fp16 z/h operands
                h = KCH // 2
                for nm in ("hi8", "xd8", "hd8"):
                    a8[nm] = acts.tile([P, KCH, BC], rdt, tag=nm, name=nm)
                    nc.sync.dma_start(a8[nm][:, :h, :],
                                      _a8srcs[nm][:, :h, :])
                    nc.sync.dma_start(a8[nm][:, h:, :],
                                      _a8srcs[nm][:, h:, :])
                hTs_l = acts.tile([P, 2 * CCH, BC], F16, tag="hTs",
                                  name="hTs")
                nc.sync.dma_start(hTs_l[:], hT16[:])
                for nm, src in (("xr16", xr16), ("xi16", xi16),
                                ("hr16", hr16), ("hi16", hi16)):
                    a16[nm] = acts.tile([P, KCH, BC], F16, tag=nm, name=nm)
                    nc.sync.dma_start(a16[nm][:], src[:])
                return hTs_l

            # weight matrices stream one per phase through two
            # alternating full-matrix slots; each load hides under the
            # previous phase's matmul chains
            wslot = [0]

            rslot = [0]

            def wphase(src, dt, nm, ncols=UC, r_gate=False):
                # gate r gets its own (small) slot triple so its loads run
                # phases ahead and the next rep's r weights prefetch during
                # this rep's z/h weight stream
                if r_gate:
                    t = acts.tile([P, KCH, ncols], dt,
                                  tag=f"wr{rslot[0] % 3}", name=nm)
                    rslot[0] += 1
                else:
                    t = acts.tile([P, KCH, ncols], dt,
                                  tag=f"w{wslot[0] % 2}", name=nm)
                    wslot[0] += 1
                # weight loads ride the ACT-engine HWDGE ring so they are
                # not FIFO-queued behind the SP-ring act/out bulk; two
                # half-loads so the first chain starts at half-landing
                h = KCH // 2
                nc.scalar.dma_start(t[:, :h, :], src[:, :h, :])
                nc.scalar.dma_start(t[:, h:, :], src[:, h:, :])
                return t

            def r_evict(ps_l, cc, rhTl, hTs):
                """rh_re/rh_im for col chunk cc from PSUM P1,P2,P3.
                3-op read-out (re = P1+P2, d = P1-P2, im = P3+d) frees the
                PSUM banks as early as possible so the next cc-pair's (and
                gate z's) matmul chains are not gated on the full chain."""
                P1, P2, P3 = ps_l
                # DVE may read only ONE PSUM operand per op; the scalar
                # engine drains P1 (the bank the next cc-pair's first
                # phases need) and DVE ops each touch one remaining bank
                s1 = yp.tile([P, BC], F32, tag="y", name=f"s1_r{cc}")
                nc.scalar.activation(s1[:], P1[:], AF.Copy)
                yre = yp.tile([P, BC], F32, tag="yd", name=f"yre_r{cc}")
                nc.vector.tensor_add(yre[:], s1[:], P2[:])
                d12 = yp.tile([P, BC], F32, tag="y", name=f"d12_r{cc}")
                nc.vector.tensor_sub(d12[:], s1[:], P2[:])
                yim = yp.tile([P, BC], F32, tag="yd", name=f"yim_r{cc}")
                nc.vector.tensor_add(yim[:], d12[:], P3[:])
                for half, y in ((0, yre), (1, yim)):
                    c = half * CCH + cc
                    a = yp.tile([P, BC], F32, tag="a", name=f"a_r{cc}{half}")
                    nc.scalar.activation(a[:], y[:], AF.Relu,
                                         bias=brTs[:, c : c + 1], scale=0.2)
                    u = yp.tile([P, BC], F16, tag="u", name=f"u_r{cc}{half}")
                    nc.scalar.activation(u[:], a[:], AF.Relu,
                                         bias=1.0, scale=-1.0)
                    # rh = hT - u*hT  (= r*h with r = 1-u)
                    nc.vector.tensor_mul(rhTl[:, c, :], u[:], hTs[:, c, :])
                    nc.vector.tensor_sub(rhTl[:, c, :], hTs[:, c, :],
                                         rhTl[:, c, :])

            def gate_r(rep, hTs, on_ccp_done):
                """fp8 DoubleRow (or fp16), output-transposed: PSUM
                [128 cols, BC] per col chunk; weights stationary.
                Two cc-pair passes x 6 single-matrix weight phases; the
                matmuls chain per PSUM bank (consecutive same-bank
                accumulation is ~90 ns/MM faster than bank rotation).
                on_ccp_done(ccp, rhTl) fires after each cc-pair's
                eviction so its AllGather half can launch mid-gate."""
                rhTl = acts.tile([P, 2 * CCH, BC], F16, tag="rhTl",
                                 name=f"rhTl_{rep}")
                phases = [("kr", "xr8", 0, 0), ("rr", "hr8", 0, 1),
                          ("ki", "xi8", 1, 0), ("ri", "hi8", 1, 1),
                          ("ks", "xd8", 2, 0), ("rs", "hd8", 2, 1)]
                nk = KCH // 2 if R_FP8 else KCH
                for ccp in range(CCH // 2):
                    # ccp1's first chains (kind 0) land in fresh banks 6,7
                    # so they start while ccp0's eviction drains banks 0-5
                    toff = 6 * ccp
                    ps = [psum.tile([P, BC], F32,
                                    tag=f"ps{(i + toff) % 8}",
                                    name=f"psr_{rep}_{ccp}_{i}")
                          for i in range(6)]  # kind*2 + cci
                    for wn, an, kind, side in phases:
                        wtl = wphase(wr[wn][ccp], rdt,
                                     f"rw{wn}_{rep}{ccp}",
                                     ncols=2 * P, r_gate=True)
                        for cci in range(2):
                            for k in range(nk):
                                if R_FP8:
                                    lhsT = wtl[:, 2 * k : 2 * k + 2,
                                               cci * P : (cci + 1) * P]
                                    rhs = a8[an][:, 2 * k : 2 * k + 2, :]
                                else:
                                    lhsT = wtl[:, k, cci * P : (cci + 1) * P]
                                    rhs = a8[an][:, k, :]
                                nc.tensor.matmul(
                                    ps[kind * 2 + cci], lhsT, rhs,
                                    start=(side == 0 and k == 0),
                                    stop=(side == 1 and k == nk - 1),
                                    perf_mode=DR if R_FP8 else None,
                                )
                    for cci in range(2):
                        ps_l = [ps[kind * 2 + cci] for kind in range(3)]
                        r_evict(ps_l, ccp * 2 + cci, rhTl, hTs)
                    on_ccp_done(ccp, rhTl)
                return rhTl

            def hs_or_tanh(dest_ap, y, gn, act_tanh):
                if act_tanh:
                    nc.scalar.activation(dest_ap, y[:], AF.Tanh)
                else:
                    a = yp.tile([P, UC], F32, tag="a", name=f"a{gn}")
                    nc.scalar.activation(a[:], y[:], AF.Relu, scale=0.2)
                    nc.scalar.activation(dest_ap, a[:], AF.Relu,
                                         bias=1.0, scale=-1.0)

            def gate_bm(rep, gn, wsrc, rstat, dest, act_tanh,
                        rko=None):
                """Batch-major gate: acts stationary fp16, weights
                moving, split by Gauss kind. Pass A: P1 (kr+rr) and P2
                (ki+ri) for all 4 m = 8 PSUM banks; evict real =
                P1+P2+b, save diff = b'+P1-P2 to SBUF. Pass B: P3
                (ks+rs), 4 banks; imag = diff+P3. One weight matrix per
                phase; matmuls chain per PSUM bank (same-bank runs are
                ~60 ns/MM faster than bank rotation); gate h's x-kind
                phases run before its recurrent phases so the collective
                hides under them."""
                r1, r2 = rstat
                # one shared slot: z's dif is fully consumed by z's own
                # pass B before gate h allocates its dif
                dif = yp.tile([P, MSUB, UC], F16, tag="dif",
                              name=f"dif{gn}_{rep}", bufs=1)

                # ---- pass A: kinds P1, P2 ----
                ps_l = [psum.tile([P, UC], F32, tag=f"ps{i}",
                                  name=f"psA{gn}_{rep}_{i}")
                        for i in range(8)]  # P1[m]=ps[m], P2[m]=ps[4+m]
                for wn, stat, bank, side in (
                    ("kr", a16["xr16"], 0, 0), ("ki", a16["xi16"], 4, 0),
                    ("rr", r1, 0, 1), ("ri", r2, 4, 1),
                ):
                    wtl = wphase(wsrc[wn], F16, f"w{gn}{wn}_{rep}")
                    # recurrent (side 1) phases consume k-chunks in
                    # gather-arrival order; PSUM accumulation commutes
                    ks_l = (rko if side == 1 and rko is not None
                            else list(range(KCH)))
                    for m in range(MSUB):
                        for ki, k in enumerate(ks_l):
                            nc.tensor.matmul(
                                ps_l[bank + m],
                                stat[:, k, m * P : (m + 1) * P],
                                wtl[:, k, :],
                                start=(side == 0 and ki == 0),
                                stop=(side == 1 and ki == KCH - 1),
                            )
                for m in range(MSUB):
                    P1, P2 = ps_l[m], ps_l[4 + m]
                    t = yp.tile([P, UC], F32, tag="y", name=f"t{gn}_{m}")
                    nc.vector.tensor_add(t[:], bt[(gn, 0)][:], P1[:])
                    # diff = (b_im - b_re) + t - P2 = b_im + P1 - P2
                    d = yp.tile([P, UC], F32, tag="yd", name=f"d{gn}_{m}")
                    nc.vector.tensor_add(d[:], bt[(gn, 1)][:], t[:])
                    nc.vector.tensor_sub(dif[:, m, :], d[:], P2[:])
                    nc.vector.tensor_add(t[:], t[:], P2[:])
                    hs_or_tanh(dest[:, m, 0:UC], t, gn, act_tanh)

                # ---- pass B: kind P3 ----
                ps_b = [psum.tile([P, UC], F32, tag=f"ps{i}",
                                  name=f"psB{gn}_{rep}_{i}")
                        for i in range(MSUB)]
                wtl = wphase(wsrc["ks"], F16, f"w{gn}ks_{rep}")
                for m in range(MSUB):
                    msl = slice(m * P, (m + 1) * P)
                    for kb in range(0, KCH, 4):
                        # x-diff stationary slices computed just-in-time
                        # (4 k-chunks per DVE op) instead of holding a
                        # resident 2 MB xd16 tile
                        xdk = yp.tile([P, 4, P], F16, tag="xdk", bufs=2,
                                      name=f"xd_{gn}{rep}{m}{kb}")
                        nc.vector.tensor_sub(
                            xdk[:], a16["xi16"][:, kb : kb + 4, msl],
                            a16["xr16"][:, kb : kb + 4, msl])
                        for j in range(4):
                            nc.tensor.matmul(
                                ps_b[m], xdk[:, j, :],
                                wtl[:, kb + j, :],
                                start=(kb + j == 0), stop=False,
                            )
                wtl = wphase(wsrc["rs"], F16, f"w{gn}rs_{rep}")
                ks_l = rko if rko is not None else list(range(KCH))
                for m in range(MSUB):
                    msl = slice(m * P, (m + 1) * P)
                    for ki, k in enumerate(ks_l):
                        rdk = yp.tile([P, P], F16, tag="rdk", bufs=3,
                                      name=f"rd_{gn}{rep}{m}{k}")
                        nc.vector.tensor_sub(rdk[:], r2[:, k, msl],
                                             r1[:, k, msl])
                        nc.tensor.matmul(
                            ps_b[m], rdk[:], wtl[:, k, :],
                            start=False, stop=(ki == KCH - 1),
                        )
                for m in range(MSUB):
                    y = yp.tile([P, UC], F32, tag="y", name=f"yi{gn}_{m}")
                    nc.vector.tensor_add(y[:], dif[:, m, :], ps_b[m][:])
                    hs_or_tanh(dest[:, m, UC : 2 * UC], y, gn, act_tanh)

            from contextlib import nullcontext

            loop_cm = (tc.For_i(0, loop_iters) if loop_iters > 1
                       else nullcontext())
            with loop_cm:
              for rep in range(repeat):
                if rep == 0:
                    load_acts_early(f"r{rep}")
                hTs = load_acts()

                # ---- gate r with a split (per cc-pair) AllGather of rh:
                # each 2-chunk half launches as soon as its eviction is
                # done, so the first half's network time hides under the
                # rest of gate r and all of gate z ----
                gparts = []

                def _collective(inb_ap, outb_ap):
                    if SKIP_COLL:
                        for q in range(NPAR):
                            nc.sync.dma_start(outb_ap[q], inb_ap)
                    else:
                        nc.gpsimd.collective_compute(
                            "AllGather",
                            mybir.AluOpType.bypass,
                            replica_groups=[[0, 1, 2, 3], [4, 5, 6, 7]],
                            ins=[inb_ap.opt()],
                            outs=[outb_ap.opt()],
                        )

                def _issue_half(ccp, rhTl):
                    inbh = dram.tile([P, 4, BC], F16, tag=f"inb{ccp}",
                                     name=f"inb{ccp}_{rep}")
                    outbh = dram.tile([NPAR, P, 4, BC], F16,
                                      tag=f"outb{ccp}",
                                      name=f"outb{ccp}_{rep}")
                    c0 = 2 * ccp
                    nc.sync.dma_start(inbh[:, 0:2, :],
                                      rhTl[:, c0 : c0 + 2, :])
                    nc.sync.dma_start(inbh[:, 2:4, :],
                                      rhTl[:, CCH + c0 : CCH + c0 + 2, :])
                    _collective(inbh[:], outbh)
                    gparts.append(outbh)

                def issue_gather(ccp, rhTl):
                    last = ccp == CCH // 2 - 1
                    if SPLIT_GATHER == 1:
                        _issue_half(ccp, rhTl)
                    elif SPLIT_GATHER == 2:
                        if last:  # both halves back-to-back at r end
                            for c in range(CCH // 2):
                                _issue_half(c, rhTl)
                    elif last:  # single gather at r end
                        inb = dram.tile([P, 2 * CCH, BC], F16,
                                        tag="inb0", name=f"inb_{rep}")
                        outb = dram.tile([NPAR, P, 2 * CCH, BC], F16,
                                         tag="outb0", name=f"outb_{rep}")
                        nc.sync.dma_start(inb[:], rhTl[:])
                        _collective(inb[:], outb)
                        gparts.append(outb)

                gate_r(rep, hTs, issue_gather)
                # prefetch the next rep's r-gate acts BEFORE the gather
                # reload DMAs: those wait on the collective semaphore and
                # would block the in-order SP queue behind them
                if rep + 1 < repeat:
                    load_acts_early(f"r{rep + 1}")
                # gathered rh K-major, aliased over hr16/hi16 (z is their
                # last reader, so the gather lands right after z); h's
                # recurrent phases consume k-chunks in gather order
                rhr = acts.tile([P, KCH, BC], F16, tag="hr16",
                                name=f"rhr_{rep}")
                rhi = acts.tile([P, KCH, BC], F16, tag="hi16",
                                name=f"rhi_{rep}")
                if SPLIT_GATHER in (1, 2):
                    for hf, outbh in enumerate(gparts):
                        for q in range(NPAR):
                            ks = slice(q * CCH + 2 * hf,
                                       q * CCH + 2 * hf + 2)
                            nc.sync.dma_start(rhr[:, ks, :],
                                              outbh[q, :, 0:2, :])
                            nc.sync.dma_start(rhi[:, ks, :],
                                              outbh[q, :, 2:4, :])
                    rec_k_order = [q * CCH + 2 * hf + j
                                   for hf in range(CCH // 2)
                                   for q in range(NPAR) for j in range(2)]
                else:
                    outb = gparts[0]
                    for q in range(NPAR):
                        qs = slice(q * CCH, (q + 1) * CCH)
                        nc.sync.dma_start(rhr[:, qs, :],
                                          outb[q, :, 0:CCH, :])
                        nc.sync.dma_start(rhi[:, qs, :],
                                          outb[q, :, CCH : 2 * CCH, :])
                    rec_k_order = None

                # z/h output buffers alias the LAST-consumed fp8 act tiles
                # (xd8/hd8, read by r's ks/rs phases) so the next rep's
                # first-needed acts (xr8/hr8/xi8) can prefetch during this
                # rep's z/h gates instead of waiting for the blend
                w_sb = acts.tile([P, MSUB, 2 * UC], F16, tag="xd8",
                                 name=f"w_sb_{rep}")
                hh_sb = acts.tile([P, MSUB, 2 * UC], F16, tag="hd8",
                                  name=f"hh_sb_{rep}")
                # h rows for the blend stream in per-m (2-deep) instead of
                # holding a third 4 MB buffer through the z/h gates
                hbm_t = {}
                for mm in range(2):
                    hbm_t[mm] = acts.tile([P, 2 * UC], F16, tag="hbm",
                                          bufs=2, name=f"hbm_{rep}_{mm}")
                    nc.sync.dma_start(hbm_t[mm][:], hbm16[:, mm, :])

                # ---- gate z (overlaps the collective) ----
                gate_bm(rep, "z", wz, (a16["hr16"], a16["hi16"]),
                        w_sb, act_tanh=False)
                # ---- gate h ----
                gate_bm(rep, "h", wh, (rhr, rhi), hh_sb,
                        act_tanh=True, rko=rec_k_order)

                # ---- blend h_new = h - w*(h - hh), per m, fp16 out ----
                for m in range(MSUB):
                    hb = hbm_t[m]
                    o1 = acts.tile([P, 2 * UC], F16, tag="hTs",
                                   name=f"o1_{rep}_{m}")
                    nc.vector.tensor_sub(o1[:], hb[:], hh_sb[:, m, :])
                    nc.vector.tensor_mul(o1[:], w_sb[:, m, :], o1[:])
                    o2 = acts.tile([P, 2 * UC], F16, tag="rhTl",
                                   name=f"o2_{rep}_{m}")
                    nc.vector.tensor_sub(o2[:], hb[:], o1[:])
                    nc.sync.dma_start(out_r[:, m, :], o2[:])
                    if m + 2 < MSUB:
                        hbm_t[m + 2] = acts.tile([P, 2 * UC], F16,
                                                 tag="hbm", bufs=2,
                                                 name=f"hbm_{rep}_{m + 2}")
                        nc.sync.dma_start(hbm_t[m + 2][:],
                                          hbm16[:, m + 2, :])

    nc.compile()
    return nc


def _pack_kmajor(a, dt):
    # (BC, K) -> (128, K//128, BC) with [p, o, b] = a[b, o*128+p]
    k = a.shape[1]
    return np.ascontiguousarray(
        a.T.reshape(k // P, P, a.shape[0]).transpose(1, 0, 2).astype(dt)
    )


def _pack_w(w, dt):
    # (2048, UC) -> (128, 16, UC) with [p, o, c] = w[o*128+p, c]
    return np.ascontiguousarray(
        w.reshape(KCH, P, UC).transpose(1, 0, 2).astype(dt)
    )


def make_in_maps(
    inputs, h_tm1, real_kernel, imaginary_kernel,
    real_recurrent_kernel, imaginary_recurrent_kernel, real_bias,
    imaginary_bias,
):
    x = np.ascontiguousarray(inputs, dtype=np.float32)
    h = np.ascontiguousarray(h_tm1, dtype=np.float32)
    rk = np.asarray(real_kernel, dtype=np.float32)
    ik = np.asarray(imaginary_kernel, dtype=np.float32)
    rr = np.asarray(real_recurrent_kernel, dtype=np.float32)
    ir = np.asarray(imaginary_recurrent_kernel, dtype=np.float32)
    rb = np.asarray(real_bias, dtype=np.float32)
    ib = np.asarray(imaginary_bias, dtype=np.float32)

    rdt = NF8 if R_FP8 else NF16
    Xr, Xi = x[:, :U], x[:, U:]
    Hr, Hi = h[:, :U], h[:, U:]
    Xd, Hd = Xi - Xr, Hi - Hr

    def _pack_w_ccp(w, dt):
        # r weights: (2048, UC) -> (2, 128, 16, 256) ccp-major
        pk = _pack_w(w, dt)
        return np.ascontiguousarray(
            np.stack([pk[:, :, c * 2 * P : (c + 1) * 2 * P]
                      for c in range(CCH // 2)])
        )

    # per-parity weight/bias packs (shared by both batch groups)
    wpk = {}
    for p in range(NPAR):
        pk = {}
        for gn, dt in (("r", rdt), ("z", NF16), ("h", NF16)):
            g = {"z": 0, "r": 1, "h": 2}[gn]
            cs = slice(g * U + p * UC, g * U + (p + 1) * UC)
            pw = _pack_w_ccp if gn == "r" else _pack_w
            pk[f"w{gn}_kr"] = pw(rk[:, cs], dt)
            pk[f"w{gn}_ki"] = pw(ik[:, cs], dt)
            pk[f"w{gn}_ks"] = pw(rk[:, cs] + ik[:, cs], dt)
            pk[f"w{gn}_rr"] = pw(rr[:, cs], dt)
            pk[f"w{gn}_ri"] = pw(ir[:, cs], dt)
            pk[f"w{gn}_rs"] = pw(rr[:, cs] + ir[:, cs], dt)
        zs = slice(p * UC, (p + 1) * UC)
        rs_ = slice(U + p * UC, U + (p + 1) * UC)
        hs_ = slice(2 * U + p * UC, 2 * U + (p + 1) * UC)
        pk["bz"] = np.stack([rb[zs] + 2.5, ib[zs] - rb[zs]]).astype(
            np.float32)
        pk["bh"] = np.stack([rb[hs_], ib[hs_] - rb[hs_]]).astype(
            np.float32)
        br = np.concatenate([rb[rs_], ib[rs_]])  # (1024,) re|im own
        pk["brT"] = np.ascontiguousarray(
            (0.2 * br + 0.5).reshape(2 * CCH, P).T.astype(np.float32)
        )
        wpk[p] = pk

    # per batch-group activation packs
    apk = {}
    for g in range(GROUPS):
        rows = slice(g * BC, (g + 1) * BC)
        apk[g] = {
            "xr8": _pack_kmajor(Xr[rows], rdt),
            "xi8": _pack_kmajor(Xi[rows], rdt),
            "xd8": _pack_kmajor(Xd[rows], rdt),
            "hr8": _pack_kmajor(Hr[rows], rdt),
            "hi8": _pack_kmajor(Hi[rows], rdt),
            "hd8": _pack_kmajor(Hd[rows], rdt),
            "xr16": _pack_kmajor(Xr[rows], NF16),
            "xi16": _pack_kmajor(Xi[rows], NF16),
            "hr16": _pack_kmajor(Hr[rows], NF16),
            "hi16": _pack_kmajor(Hi[rows], NF16),
        }

    in_maps = []
    for c in range(N_CORES):
        g, p = c // NPAR, c % NPAR
        rows = slice(g * BC, (g + 1) * BC)
        hcat = np.concatenate(
            [h[rows, p * UC : (p + 1) * UC],
             h[rows, U + p * UC : U + (p + 1) * UC]], axis=1
        )  # (BC, 1024) own re|im
        hT = np.ascontiguousarray(
            hcat.T.reshape(2 * CCH, P, BC).transpose(1, 0, 2).astype(NF16)
        )
        hbm = np.ascontiguousarray(
            hcat.reshape(MSUB, P, 2 * UC).transpose(1, 0, 2).astype(NF16)
        )
        m = {"hT16": hT, "hbm16": hbm}
        m.update(apk[g])
        m.update(wpk[p])
        in_maps.append(m)
    return in_maps


def scatter_out(results):
    h_new = np.empty((B, 2 * U), dtype=np.float32)
    for c in range(N_CORES):
        g, p = c // NPAR, c % NPAR
        rows = slice(g * BC, (g + 1) * BC)
        o = results[c]["out"]
        h_new[rows, p * UC : (p + 1) * UC] = o[:, :UC]
        h_new[rows, U + p * UC : U + (p + 1) * UC] = o[:, UC:]
    return h_new


def _build_nc_retry(repeat=1, loop_iters=1, attempts=4):
    # Tile's scheduler very occasionally reports a spurious deadlock on a
    # valid graph (ordering is not fully deterministic); retry a few times.
    last = None
    for _ in range(attempts):
        try:
            return _build_nc(repeat=repeat, loop_iters=loop_iters)
        except Exception as e:  # noqa: BLE001
            if "Deadlock" not in type(e).__name__ + str(e):
                raise
            last = e
    raise last


def kernel(
    inputs,
    h_tm1,
    real_kernel,
    imaginary_kernel,
    real_recurrent_kernel,
    imaginary_recurrent_kernel,
    real_bias,
    imaginary_bias,
):
    if "nc" not in _CACHE:
        _CACHE["nc"] = _build_nc_retry()
    nc = _CACHE["nc"]
    in_maps = make_in_maps(
        inputs, h_tm1, real_kernel, imaginary_kernel,
        real_recurrent_kernel, imaginary_recurrent_kernel, real_bias,
        imaginary_bias,
    )
    res = run_bass_kernel_spmd(nc, in_maps, core_ids=list(range(N_CORES)))
    return scatter_out(res.results)

